# revision 10
# baseline (speedup 1.0000x reference)
"""Trainium2 Bass kernel for nn_MHAttentionMap (scrambled-reshape variant).

Math (derived from the reference's permute/reshape semantics):
    ql = q @ Wq^T + bq                  # [A, B, H]
    kl = k @ Wk^T + bk                  # [B, H]
    logits[alpha, m] = fact * sum_a ql[a, alpha, m] * kl[a, m]   # m in [0, H)
    out[alpha, beta, n] = softmax_n(logits[alpha, 8*beta + n])   # groups of 8

Sharding: data-parallel over alpha (q's second axis), J=32 columns per core,
no collectives. The dominant GEMM runs on PE in bf16 with f32 PSUM
accumulation; the tiny kl projection is folded on the host.

End-to-end latency design (the metric is wall-clock of kernel() in a fresh
process; device exec is ~ms while host/compile/transfer costs dominate):
  - The Bass program uses a For_i hardware loop over 16 a-groups, so the
    program is ~350 instructions instead of ~8500: fast build, fast
    compile, small NEFF, fast device graph load.
  - Inputs ship as bf16 (q: 268MB instead of 537MB) — the axon tunnel
    sustains only ~90MB/s, so bytes shipped dominate.
  - A background thread started at import builds + compiles the program
    and issues a warm-up call on device-resident zeros, so the NEFF load
    overlaps the host-side prep and input transfer.
  - kernel() preps per-core shards and device_puts them as they become
    ready, then invokes the compiled executable on device-resident arrays.
"""

import threading
import numpy as np

import concourse.bass as bass
import concourse.mybir as mybir
import concourse.tile_sem_assignment as _tsa
from concourse.bass import ds
from concourse.tile import TileContext

_tsa.NUM_HWDGE_SEMS = 1  # all nc.sync DMAs share one FIFO ring/semaphore

A = 256          # q leading axis (contracted in the output)
B = 256          # q second axis (sharded)
H = 2048         # hidden
NH = 8           # heads (softmax group)
NCORES = 8
J = B // NCORES  # 32 alpha columns per core
HC = H // 128    # 16 contraction chunks
MT = H // 128    # 16 m tiles
AGN = 16         # a-groups
AGS = A // AGN   # 16 a per group
FREE = AGS * J   # 512 matmul free size
FACT = float((H / NH) ** -0.5)

F32 = mybir.dt.float32
BF16 = mybir.dt.bfloat16
MULT = mybir.AluOpType.mult
ADD = mybir.AluOpType.add

IN_NAMES = ["qG", "WqT", "klT", "bqk"]  # must match allocation order in build()


def build():
    nc = bass.Bass()
    qG = nc.dram_tensor("qG", [AGN, 128, HC * FREE], BF16, kind="ExternalInput")
    WqT = nc.dram_tensor("WqT", [H, H], BF16, kind="ExternalInput")
    klT = nc.dram_tensor("klT", [AGN, 128, MT, AGS], F32, kind="ExternalInput")
    bqk = nc.dram_tensor("bqk", [128, MT], F32, kind="ExternalInput")
    out = nc.dram_tensor("out", [J, H], F32, kind="ExternalOutput")

    ident_d = nc.inline_tensor(np.eye(128, dtype=np.float32), name="ident")
    g_np = np.kron(np.eye(16, dtype=np.float32), np.ones((8, 1), np.float32))
    g_d = nc.inline_tensor(g_np, name="gmat")                            # [128, 16]
    gt_d = nc.inline_tensor(np.ascontiguousarray(g_np.T), name="gtmat")  # [16, 128]

    with TileContext(nc) as tc:
        with (
            tc.tile_pool(name="const", bufs=1) as cpool,
            tc.tile_pool(name="qb", bufs=1) as qpool,
            tc.tile_pool(name="acc", bufs=1) as apool,
            tc.tile_pool(name="ework", bufs=2) as epool,
            tc.tile_pool(name="mpsum", bufs=8, space="PSUM") as mpsum,
        ):
            ident_sb = cpool.tile([128, 128], F32, name="ident_sb")
            nc.sync.dma_start(ident_sb[:], ident_d[:])
            g_sb = cpool.tile([128, 16], F32, name="g_sb")
            nc.sync.dma_start(g_sb[:], g_d[:])
            gt_sb = cpool.tile([16, 128], F32, name="gt_sb")
            nc.sync.dma_start(gt_sb[:], gt_d[:])

            wq_sb = cpool.tile([128, HC, H], BF16, name="wq_sb")
            nc.sync.dma_start(wq_sb[:], WqT[:].rearrange("(c p) m -> p c m", p=128))
            bqk_sb = cpool.tile([128, MT], F32, name="bqk_sb")
            nc.sync.dma_start(bqk_sb[:], bqk[:])

            s_all = apool.tile([128, MT, J], F32, name="s_all")
            nc.vector.memset(s_all[:], 0.0)

            with tc.For_i(0, AGN, 1) as ag:
                qblk = qpool.tile([128, HC * FREE], BF16, name="qblk")
                nc.sync.dma_start(qblk[:], qG[ds(ag, 1), :, :])
                klcur = qpool.tile([128, MT, AGS], F32, name="klcur")
                nc.sync.dma_start(klcur[:], klT[ds(ag, 1), :, :, :])
                for mt in range(MT):
                    ps = mpsum.tile([128, FREE], F32, name="ps", tag="ps")
                    for hc in range(HC):
                        nc.tensor.matmul(
                            ps[:],
                            wq_sb[:, hc, mt * 128 : (mt + 1) * 128],
                            qblk[:, hc * FREE : (hc + 1) * FREE],
                            start=(hc == 0),
                            stop=(hc == HC - 1),
                        )
                    # e[p, j, al] = ps[p, al*J+j] * klcur[p, mt, al]
                    e = epool.tile([128, J, AGS], F32, name="e", tag="e")
                    nc.vector.tensor_tensor(
                        e[:],
                        ps[:].rearrange("p (al j) -> p j al", j=J),
                        klcur[:, mt, :].unsqueeze(1).broadcast_to([128, J, AGS]),
                        op=MULT,
                    )
                    r = epool.tile([128, J], F32, name="r", tag="r")
                    nc.vector.tensor_reduce(
                        r[:], e[:], axis=mybir.AxisListType.X, op=ADD
                    )
                    nc.vector.tensor_tensor(
                        s_all[:, mt, :], r[:], s_all[:, mt, :], op=ADD
                    )

            # bias fold: s[p, mt, j] += bqk[p, mt]
            nc.vector.tensor_tensor(
                s_all[:],
                bqk_sb[:].unsqueeze(-1).broadcast_to([128, MT, J]),
                s_all[:],
                op=ADD,
            )

            # softmax over groups of 8 partitions; logits ~ N(0,1) so exp
            # without max-subtraction is safe in f32.
            e_all = apool.tile([128, MT, J], F32, name="e_all")
            nc.scalar.activation(e_all[:], s_all[:], mybir.ActivationFunctionType.Exp)
            zp = mpsum.tile([16, MT * J], F32, name="zp", tag="ps")
            nc.tensor.matmul(
                zp[:], g_sb[:], e_all[:].rearrange("p mt j -> p (mt j)"),
                start=True, stop=True,
            )
            rz_sb = apool.tile([16, MT * J], F32, name="rz_sb")
            nc.vector.reciprocal(rz_sb[:], zp[:])
            rp = mpsum.tile([128, MT * J], F32, name="rp", tag="ps")
            nc.tensor.matmul(rp[:], gt_sb[:], rz_sb[:], start=True, stop=True)
            w_all = apool.tile([128, MT, J], F32, name="w_all")
            nc.vector.tensor_tensor(
                w_all[:], e_all[:],
                rp[:].rearrange("p (mt j) -> p mt j", j=J),
                op=MULT,
            )

            # transpose [m, j] -> [j, m] and store
            wT = apool.tile([J, MT, 128], F32, name="wT")
            for tpi in range(4):
                tp = mpsum.tile([J, 4, 128], F32, name="tp", tag="ps")
                for k4 in range(4):
                    mtg = tpi * 4 + k4
                    nc.tensor.transpose(tp[:, k4, :], w_all[:, mtg, :], ident_sb[:])
                nc.vector.tensor_copy(wT[:, tpi * 4 : (tpi + 1) * 4, :], tp[:])
            nc.sync.dma_start(out[:], wT[:])

    _hoist_waits(nc)
    return nc


def _hoist_waits(nc):
    """This walrus build allows only one semaphore wait per TPB/DMA
    instruction. Hoist all-but-one wait of each instruction onto standalone
    EventSemaphore sync ops on the same engine, issued immediately before —
    the engine sequencer executes in order, so semantics are unchanged."""
    import bass_rust

    skip = ("InstEventSemaphore", "InstCall", "InstISA")
    for f in nc.m.functions:
        for bb in f.blocks:
            out = []
            for inst in bb.instructions:
                si = inst.sync_info
                if (
                    si is not None
                    and si.on_wait
                    and len(si.on_wait) > 1
                    and type(inst).__name__ not in skip
                ):
                    waits = list(si.on_wait)
                    for w in waits[:-1]:
                        es = mybir.InstEventSemaphore(
                            name=f"{inst.name}-w{len(out)}",
                            engine=inst.engine,
                            sync_info=bass_rust.SyncInfo(on_wait=[w], on_update=[]),
                        )
                        out.append(es)
                    si.on_wait = waits[-1:]
                out.append(inst)
            bb.instructions = out


# ---------------------------------------------------------------------------
# Host-side runner: compiled-executable cache + background warm-up.
# ---------------------------------------------------------------------------

_ST: dict = {}
_DEV_READY = threading.Event()
_COMPILED_READY = threading.Event()

import os as _os
import sys as _sys
import time as _time

_T0 = _time.time()
_DEBUG = bool(_os.environ.get("KERNEL_DEBUG"))


def _dbg(msg):
    if _DEBUG:
        print(f"[kernel +{_time.time()-_T0:6.2f}s] {msg}", file=_sys.stderr, flush=True)


def _input_specs():
    """(name, per-core shape, numpy dtype) in executable parameter order."""
    import ml_dtypes

    bf16 = np.dtype(ml_dtypes.bfloat16)
    return [
        ("qG", (AGN, 128, HC * FREE), bf16),
        ("WqT", (H, H), bf16),
        ("klT", (AGN, 128, MT, AGS), np.dtype(np.float32)),
        ("bqk", (128, MT), np.dtype(np.float32)),
    ]


def _bg_compile():
    try:
        import jax
        from jax.sharding import Mesh, PartitionSpec, NamedSharding
        from jax.experimental.shard_map import shard_map
        import concourse.bass2jax as b2j

        _dbg("bg: jax imported")
        devices = jax.devices()[:NCORES]
        mesh = Mesh(np.asarray(devices), ("core",))
        sh = NamedSharding(mesh, PartitionSpec("core"))
        _ST["devices"] = devices
        _ST["mesh"] = mesh
        _ST["sharding"] = sh
        _DEV_READY.set()
        _dbg("bg: devices ready")

        b2j.install_neuronx_cc_hook()
        nc = build()
        _dbg("bg: bass built")
        assert nc.dbg_addr is None
        partition_name = (
            nc.partition_id_tensor.name if nc.partition_id_tensor else None
        )

        # Recover the executable's input/output interface from allocations.
        in_names, out_names, out_avals = [], [], []
        for alloc in nc.m.functions[0].allocations:
            if not isinstance(alloc, mybir.MemoryLocationSet):
                continue
            name = alloc.memorylocations[0].name
            if alloc.kind == "ExternalInput":
                if name != partition_name:
                    in_names.append(name)
            elif alloc.kind == "ExternalOutput":
                out_names.append(name)
                out_avals.append(
                    jax.core.ShapedArray(
                        tuple(alloc.tensor_shape), mybir.dt.np(alloc.dtype)
                    )
                )
        assert in_names == IN_NAMES, in_names
        assert out_names == ["out"], out_names
        n_params = len(in_names)
        all_names = in_names + out_names
        if partition_name is not None:
            all_names.append(partition_name)
        all_names = tuple(all_names)
        donate = tuple(range(n_params, n_params + len(out_names)))

        def _body(*args):
            operands = list(args)
            if partition_name is not None:
                operands.append(b2j.partition_id_tensor())
            outs = b2j._bass_exec_p.bind(
                *operands,
                out_avals=tuple(out_avals),
                in_names=all_names,
                out_names=tuple(out_names),
                lowering_input_output_aliases=(),
                sim_require_finite=True,
                sim_require_nnan=True,
                nc=nc,
            )
            return tuple(outs)

        jf = jax.jit(
            shard_map(
                _body,
                mesh=mesh,
                in_specs=(PartitionSpec("core"),) * (n_params + len(out_names)),
                out_specs=(PartitionSpec("core"),) * len(out_names),
                check_rep=False,
            ),
            donate_argnums=donate,
            keep_unused=True,
        )

        specs = _input_specs()
        gshapes = [(NCORES * s[0], *s[1:]) for _, s, _ in specs]
        gdtypes = [d for _, _, d in specs]
        out_gshape = (NCORES * J, H)
        abstract = [
            jax.ShapeDtypeStruct(s, d, sharding=sh)
            for s, d in zip(gshapes, gdtypes)
        ] + [jax.ShapeDtypeStruct(out_gshape, np.float32, sharding=sh)]
        lowered = jf.lower(*abstract)
        _dbg("bg: lowered")
        compiled = lowered.compile()
        _dbg("bg: compiled")

        # Device-resident zeros for the warm-up call and for the real call's
        # donated output buffer.
        zfn = jax.jit(
            lambda: tuple(
                [jax.numpy.zeros(s, d) for s, d in zip(gshapes, gdtypes)]
                + [jax.numpy.zeros(out_gshape, np.float32)]
            ),
            out_shardings=(sh,) * (len(gshapes) + 1),
        )
        warm = zfn()
        out_zeros = zfn()[-1]
        jax.block_until_ready(warm)
        _dbg("bg: device zeros ready")
        # Warm-up: ships + loads the NEFF on all cores, establishes comms.
        jax.block_until_ready(compiled(*warm))
        _dbg("bg: warm call done")
        _ST["compiled"] = compiled
        _ST["out_zeros"] = out_zeros
    except Exception as exc:  # noqa: BLE001
        _ST["err"] = exc
    finally:
        _DEV_READY.set()
        _COMPILED_READY.set()


_BG = threading.Thread(target=_bg_compile, daemon=True)
_BG.start()


def _prep_small(k, Wq, bq, Wk, bk):
    import ml_dtypes

    bf16 = np.dtype(ml_dtypes.bfloat16)
    WqTb = np.ascontiguousarray(np.asarray(Wq, np.float32).T).astype(bf16)
    klF = (
        np.asarray(k, np.float32) @ np.asarray(Wk, np.float32).T
        + np.asarray(bk, np.float32)
    ) * np.float32(FACT)                                          # [A, H]
    klT4 = np.ascontiguousarray(
        klF.reshape(AGN, AGS, MT, 128).transpose(0, 3, 2, 1)
    )                                                             # [AGN,128,MT,AGS]
    bqk_m = np.asarray(bq, np.float32) * klF.sum(axis=0)
    bqk = np.ascontiguousarray(bqk_m.reshape(MT, 128).T)          # [128, MT]
    return WqTb, klT4, bqk


def _prep_q_core(q, c):
    import ml_dtypes

    bf16 = np.dtype(ml_dtypes.bfloat16)
    qc = q[:, c * J : (c + 1) * J, :]
    return (
        qc.reshape(AGN, AGS, J, HC, 128)
        .transpose(0, 4, 3, 1, 2)
        .astype(bf16)
        .reshape(AGN, 128, HC * FREE)
    )


def kernel(q, k, Wq, bq, Wk, bk):
    import jax

    _dbg("kernel: called")
    q = np.asarray(q, dtype=np.float32)

    _DEV_READY.wait()
    _dbg("kernel: devices ready")
    if "devices" not in _ST:
        raise RuntimeError(f"jax init failed: {_ST.get('err')}")
    devices = _ST["devices"]
    sh = _ST["sharding"]
    specs = _input_specs()

    # Small arrays first (cheap to prep); ship while q is being prepped.
    WqTb, klT4, bqk = _prep_small(k, Wq, bq, Wk, bk)
    _dbg("kernel: small prepped")
    shards: dict = {n: [None] * NCORES for n in IN_NAMES}
    for c in range(NCORES):
        shards["WqT"][c] = jax.device_put(WqTb, devices[c])
        shards["klT"][c] = jax.device_put(klT4, devices[c])
        shards["bqk"][c] = jax.device_put(bqk, devices[c])
    _dbg("kernel: small puts issued")
    for c in range(NCORES):
        shards["qG"][c] = jax.device_put(_prep_q_core(q, c), devices[c])
        _dbg(f"kernel: qG put {c} issued")

    gargs = []
    for name, pshape, pdtype in specs:
        garr = jax.make_array_from_single_device_arrays(
            (NCORES * pshape[0], *pshape[1:]), sh, shards[name]
        )
        gargs.append(garr)
    jax.block_until_ready(gargs)
    _dbg("kernel: transfers complete")

    _COMPILED_READY.wait()
    _dbg("kernel: compiled ready")
    if "compiled" not in _ST:
        # Background compile failed — fall back to the stock runner.
        from concourse.bass_utils import run_bass_kernel_spmd

        in_maps = [
            {"qG": np.asarray(shards["qG"][c]), "WqT": WqTb, "klT": klT4,
             "bqk": bqk}
            for c in range(NCORES)
        ]
        nc = build()
        res = run_bass_kernel_spmd(nc, in_maps, core_ids=list(range(NCORES)))
        outs = [r["out"] for r in res.results]
        return np.concatenate(outs, axis=0).reshape(A, B, NH, 1, 1)

    compiled = _ST["compiled"]
    outs = compiled(*gargs, _ST["out_zeros"])
    jax.block_until_ready(outs)
    _dbg("kernel: exec done")
    out_np = np.asarray(outs[0])                                  # [B, H]
    _dbg("kernel: fetched")
    return out_np.reshape(A, B, NH, 1, 1)


# revision 14
# speedup vs baseline: 1.9436x; 1.9436x over previous
"""Trainium2 Bass kernel for nn_MHAttentionMap (scrambled-reshape variant).

Math (derived from the reference's permute/reshape semantics):
    ql = q @ Wq^T + bq                  # [A, B, H]
    kl = k @ Wk^T + bk                  # [B, H]
    logits[alpha, m] = fact * sum_a ql[a, alpha, m] * kl[a, m]   # m in [0, H)
    out[alpha, beta, n] = softmax_n(logits[alpha, 8*beta + n])   # groups of 8

Sharding: data-parallel over alpha (q's second axis), J=32 columns per core,
no collectives. The dominant GEMM runs on PE in bf16 with f32 PSUM
accumulation; the tiny kl projection is folded on the host.

End-to-end latency design (the metric is wall-clock of kernel() in a fresh
process; device exec is ~ms while host/compile/transfer costs dominate):
  - The Bass program uses a For_i hardware loop over 16 a-groups, so the
    program is ~350 instructions instead of ~8500: fast build, fast
    compile, small NEFF, fast device graph load.
  - Inputs ship as bf16 (q: 268MB instead of 537MB) — the axon tunnel
    sustains only ~90MB/s, so bytes shipped dominate.
  - A background thread started at import builds + compiles the program
    and issues a warm-up call on device-resident zeros, so the NEFF load
    overlaps the host-side prep and input transfer.
  - kernel() preps per-core shards and device_puts them as they become
    ready, then invokes the compiled executable on device-resident arrays.
"""

import threading
import numpy as np

import concourse.bass as bass
import concourse.mybir as mybir
import concourse.tile_sem_assignment as _tsa
from concourse.bass import ds
from concourse.tile import TileContext

_tsa.NUM_HWDGE_SEMS = 1  # all nc.sync DMAs share one FIFO ring/semaphore

A = 256          # q leading axis (contracted in the output)
B = 256          # q second axis (sharded)
H = 2048         # hidden
NH = 8           # heads (softmax group)
NCORES = 8
J = B // NCORES  # 32 alpha columns per core
HC = H // 128    # 16 contraction chunks
MT = H // 128    # 16 m tiles
AGN = 16         # a-groups
AGS = A // AGN   # 16 a per group
FREE = AGS * J   # 512 matmul free size
FACT = float((H / NH) ** -0.5)

F32 = mybir.dt.float32
BF16 = mybir.dt.bfloat16
MULT = mybir.AluOpType.mult
ADD = mybir.AluOpType.add

IN_NAMES = ["qG", "WqT", "klT", "bqk"]  # must match allocation order in build()


def build():
    nc = bass.Bass()
    qG = nc.dram_tensor("qG", [AGN, 128, HC * FREE], BF16, kind="ExternalInput")
    WqT = nc.dram_tensor("WqT", [H, H], BF16, kind="ExternalInput")
    klT = nc.dram_tensor("klT", [AGN, 128, MT, AGS], F32, kind="ExternalInput")
    bqk = nc.dram_tensor("bqk", [128, MT], F32, kind="ExternalInput")
    out = nc.dram_tensor("out", [J, H], F32, kind="ExternalOutput")

    ident_d = nc.inline_tensor(np.eye(128, dtype=np.float32), name="ident")
    g_np = np.kron(np.eye(16, dtype=np.float32), np.ones((8, 1), np.float32))
    g_d = nc.inline_tensor(g_np, name="gmat")                            # [128, 16]
    gt_d = nc.inline_tensor(np.ascontiguousarray(g_np.T), name="gtmat")  # [16, 128]

    with TileContext(nc) as tc:
        with (
            tc.tile_pool(name="const", bufs=1) as cpool,
            tc.tile_pool(name="qb", bufs=1) as qpool,
            tc.tile_pool(name="acc", bufs=1) as apool,
            tc.tile_pool(name="ework", bufs=2) as epool,
            tc.tile_pool(name="mpsum", bufs=8, space="PSUM") as mpsum,
        ):
            ident_sb = cpool.tile([128, 128], F32, name="ident_sb")
            nc.sync.dma_start(ident_sb[:], ident_d[:])
            g_sb = cpool.tile([128, 16], F32, name="g_sb")
            nc.sync.dma_start(g_sb[:], g_d[:])
            gt_sb = cpool.tile([16, 128], F32, name="gt_sb")
            nc.sync.dma_start(gt_sb[:], gt_d[:])

            wq_sb = cpool.tile([128, HC, H], BF16, name="wq_sb")
            nc.sync.dma_start(wq_sb[:], WqT[:].rearrange("(c p) m -> p c m", p=128))
            bqk_sb = cpool.tile([128, MT], F32, name="bqk_sb")
            nc.sync.dma_start(bqk_sb[:], bqk[:])

            s_all = apool.tile([128, MT, J], F32, name="s_all")
            nc.vector.memset(s_all[:], 0.0)

            with tc.For_i(0, AGN, 1) as ag:
                qblk = qpool.tile([128, HC * FREE], BF16, name="qblk")
                nc.sync.dma_start(qblk[:], qG[ds(ag, 1), :, :])
                klcur = qpool.tile([128, MT, AGS], F32, name="klcur")
                nc.sync.dma_start(klcur[:], klT[ds(ag, 1), :, :, :])
                for mt in range(MT):
                    ps = mpsum.tile([128, FREE], F32, name="ps", tag="ps")
                    for hc in range(HC):
                        nc.tensor.matmul(
                            ps[:],
                            wq_sb[:, hc, mt * 128 : (mt + 1) * 128],
                            qblk[:, hc * FREE : (hc + 1) * FREE],
                            start=(hc == 0),
                            stop=(hc == HC - 1),
                        )
                    # e[p, j, al] = ps[p, al*J+j] * klcur[p, mt, al]
                    e = epool.tile([128, J, AGS], F32, name="e", tag="e")
                    nc.vector.tensor_tensor(
                        e[:],
                        ps[:].rearrange("p (al j) -> p j al", j=J),
                        klcur[:, mt, :].unsqueeze(1).broadcast_to([128, J, AGS]),
                        op=MULT,
                    )
                    r = epool.tile([128, J], F32, name="r", tag="r")
                    nc.vector.tensor_reduce(
                        r[:], e[:], axis=mybir.AxisListType.X, op=ADD
                    )
                    nc.vector.tensor_tensor(
                        s_all[:, mt, :], r[:], s_all[:, mt, :], op=ADD
                    )

            # bias fold: s[p, mt, j] += bqk[p, mt]
            nc.vector.tensor_tensor(
                s_all[:],
                bqk_sb[:].unsqueeze(-1).broadcast_to([128, MT, J]),
                s_all[:],
                op=ADD,
            )

            # softmax over groups of 8 partitions; logits ~ N(0,1) so exp
            # without max-subtraction is safe in f32.
            e_all = apool.tile([128, MT, J], F32, name="e_all")
            nc.scalar.activation(e_all[:], s_all[:], mybir.ActivationFunctionType.Exp)
            zp = mpsum.tile([16, MT * J], F32, name="zp", tag="ps")
            nc.tensor.matmul(
                zp[:], g_sb[:], e_all[:].rearrange("p mt j -> p (mt j)"),
                start=True, stop=True,
            )
            rz_sb = apool.tile([16, MT * J], F32, name="rz_sb")
            nc.vector.reciprocal(rz_sb[:], zp[:])
            rp = mpsum.tile([128, MT * J], F32, name="rp", tag="ps")
            nc.tensor.matmul(rp[:], gt_sb[:], rz_sb[:], start=True, stop=True)
            w_all = apool.tile([128, MT, J], F32, name="w_all")
            nc.vector.tensor_tensor(
                w_all[:], e_all[:],
                rp[:].rearrange("p (mt j) -> p mt j", j=J),
                op=MULT,
            )

            # transpose [m, j] -> [j, m] and store
            wT = apool.tile([J, MT, 128], F32, name="wT")
            for tpi in range(4):
                tp = mpsum.tile([J, 4, 128], F32, name="tp", tag="ps")
                for k4 in range(4):
                    mtg = tpi * 4 + k4
                    nc.tensor.transpose(tp[:, k4, :], w_all[:, mtg, :], ident_sb[:])
                nc.vector.tensor_copy(wT[:, tpi * 4 : (tpi + 1) * 4, :], tp[:])
            nc.sync.dma_start(out[:], wT[:])

    _hoist_waits(nc)
    return nc


def _hoist_waits(nc):
    """This walrus build allows only one semaphore wait per TPB/DMA
    instruction. Hoist all-but-one wait of each instruction onto standalone
    EventSemaphore sync ops on the same engine, issued immediately before —
    the engine sequencer executes in order, so semantics are unchanged."""
    import bass_rust

    skip = ("InstEventSemaphore", "InstCall", "InstISA")
    for f in nc.m.functions:
        for bb in f.blocks:
            out = []
            for inst in bb.instructions:
                si = inst.sync_info
                if (
                    si is not None
                    and si.on_wait
                    and len(si.on_wait) > 1
                    and type(inst).__name__ not in skip
                ):
                    waits = list(si.on_wait)
                    for w in waits[:-1]:
                        es = mybir.InstEventSemaphore(
                            name=f"{inst.name}-w{len(out)}",
                            engine=inst.engine,
                            sync_info=bass_rust.SyncInfo(on_wait=[w], on_update=[]),
                        )
                        out.append(es)
                    si.on_wait = waits[-1:]
                out.append(inst)
            bb.instructions = out


# ---------------------------------------------------------------------------
# Host-side runner: compiled-executable cache + background warm-up.
# ---------------------------------------------------------------------------

_ST: dict = {}
_DEV_READY = threading.Event()
_COMPILED_READY = threading.Event()

import os as _os
import sys as _sys
import time as _time

_T0 = _time.time()
_DEBUG = bool(_os.environ.get("KERNEL_DEBUG"))


def _dbg(msg):
    if _DEBUG:
        print(f"[kernel +{_time.time()-_T0:6.2f}s] {msg}", file=_sys.stderr, flush=True)


def _input_specs():
    """(name, per-core shape, numpy dtype) in executable parameter order."""
    import ml_dtypes

    bf16 = np.dtype(ml_dtypes.bfloat16)
    return [
        ("qG", (AGN, 128, HC * FREE), bf16),
        ("WqT", (H, H), bf16),
        ("klT", (AGN, 128, MT, AGS), np.dtype(np.float32)),
        ("bqk", (128, MT), np.dtype(np.float32)),
    ]


def _bg_compile():
    try:
        import jax
        from jax.sharding import Mesh, PartitionSpec, NamedSharding
        from jax.experimental.shard_map import shard_map
        import concourse.bass2jax as b2j

        _dbg("bg: jax imported")
        devices = jax.devices()[:NCORES]
        mesh = Mesh(np.asarray(devices), ("core",))
        sh = NamedSharding(mesh, PartitionSpec("core"))
        _ST["devices"] = devices
        _ST["mesh"] = mesh
        _ST["sharding"] = sh
        _DEV_READY.set()
        _dbg("bg: devices ready")

        b2j.install_neuronx_cc_hook()
        nc = build()
        _dbg("bg: bass built")
        assert nc.dbg_addr is None
        partition_name = (
            nc.partition_id_tensor.name if nc.partition_id_tensor else None
        )

        # Recover the executable's input/output interface from allocations.
        in_names, out_names, out_avals = [], [], []
        for alloc in nc.m.functions[0].allocations:
            if not isinstance(alloc, mybir.MemoryLocationSet):
                continue
            name = alloc.memorylocations[0].name
            if alloc.kind == "ExternalInput":
                if name != partition_name:
                    in_names.append(name)
            elif alloc.kind == "ExternalOutput":
                out_names.append(name)
                out_avals.append(
                    jax.core.ShapedArray(
                        tuple(alloc.tensor_shape), mybir.dt.np(alloc.dtype)
                    )
                )
        assert in_names == IN_NAMES, in_names
        assert out_names == ["out"], out_names
        n_params = len(in_names)
        all_names = in_names + out_names
        if partition_name is not None:
            all_names.append(partition_name)
        all_names = tuple(all_names)
        donate = tuple(range(n_params, n_params + len(out_names)))

        def _body(*args):
            operands = list(args)
            if partition_name is not None:
                operands.append(b2j.partition_id_tensor())
            outs = b2j._bass_exec_p.bind(
                *operands,
                out_avals=tuple(out_avals),
                in_names=all_names,
                out_names=tuple(out_names),
                lowering_input_output_aliases=(),
                sim_require_finite=True,
                sim_require_nnan=True,
                nc=nc,
            )
            return tuple(outs)

        jf = jax.jit(
            shard_map(
                _body,
                mesh=mesh,
                in_specs=(PartitionSpec("core"),) * (n_params + len(out_names)),
                out_specs=(PartitionSpec("core"),) * len(out_names),
                check_rep=False,
            ),
            donate_argnums=donate,
            keep_unused=True,
        )

        specs = _input_specs()
        gshapes = [(NCORES * s[0], *s[1:]) for _, s, _ in specs]
        gdtypes = [d for _, _, d in specs]
        out_gshape = (NCORES * J, H)
        abstract = [
            jax.ShapeDtypeStruct(s, d, sharding=sh)
            for s, d in zip(gshapes, gdtypes)
        ] + [jax.ShapeDtypeStruct(out_gshape, np.float32, sharding=sh)]
        lowered = jf.lower(*abstract)
        _dbg("bg: lowered")
        compiled = lowered.compile()
        _dbg("bg: compiled")

        # On-device replication programs: WqT/klT ship once (row-sharded)
        # and are all-gathered into the per-core-replicated global layout
        # the executable expects.
        jnp = jax.numpy
        import ml_dtypes

        bf16 = np.dtype(ml_dtypes.bfloat16)
        _ST["tile_wq"] = (
            jax.jit(lambda w: jnp.tile(w, (NCORES, 1)), out_shardings=sh)
            .lower(jax.ShapeDtypeStruct((H, H), bf16, sharding=sh))
            .compile()
        )
        _ST["tile_kl"] = (
            jax.jit(lambda w: jnp.tile(w, (NCORES, 1, 1, 1)), out_shardings=sh)
            .lower(
                jax.ShapeDtypeStruct((AGN, 128, MT, AGS), np.float32, sharding=sh)
            )
            .compile()
        )
        _dbg("bg: tile jits compiled")
        _ST["compiled"] = compiled
    except Exception as exc:  # noqa: BLE001
        _ST["err"] = exc
    finally:
        _DEV_READY.set()
        _COMPILED_READY.set()


_BG = threading.Thread(target=_bg_compile, daemon=True)
_BG.start()


def _prep_small(k, Wq, bq, Wk, bk):
    import ml_dtypes

    bf16 = np.dtype(ml_dtypes.bfloat16)
    WqTb = np.ascontiguousarray(np.asarray(Wq, np.float32).T).astype(bf16)
    klF = (
        np.asarray(k, np.float32) @ np.asarray(Wk, np.float32).T
        + np.asarray(bk, np.float32)
    ) * np.float32(FACT)                                          # [A, H]
    klT4 = np.ascontiguousarray(
        klF.reshape(AGN, AGS, MT, 128).transpose(0, 3, 2, 1)
    )                                                             # [AGN,128,MT,AGS]
    bqk_m = np.asarray(bq, np.float32) * klF.sum(axis=0)
    bqk = np.ascontiguousarray(bqk_m.reshape(MT, 128).T)          # [128, MT]
    return WqTb, klT4, bqk


def _prep_q_core(q, c):
    import ml_dtypes

    bf16 = np.dtype(ml_dtypes.bfloat16)
    qc = q[:, c * J : (c + 1) * J, :]
    return (
        qc.reshape(AGN, AGS, J, HC, 128)
        .transpose(0, 4, 3, 1, 2)
        .astype(bf16)
        .reshape(AGN, 128, HC * FREE)
    )


def kernel(q, k, Wq, bq, Wk, bk):
    import jax

    _dbg("kernel: called")
    q = np.asarray(q, dtype=np.float32)

    # Small arrays are cheap to prep and ship; do them first so the device
    # side work (all-gathers) pipelines under the long qG transfer.
    WqTb, klT4, bqk = _prep_small(k, Wq, bq, Wk, bk)
    _dbg("kernel: small prepped")

    _DEV_READY.wait()
    _dbg("kernel: devices ready")
    if "devices" not in _ST:
        raise RuntimeError(f"jax init failed: {_ST.get('err')}")
    devices = _ST["devices"]
    sh = _ST["sharding"]

    # WqT/klT: ship once row-sharded; replicate on device once compiled.
    wq_sharded = jax.device_put(WqTb, sh)
    kl_sharded = jax.device_put(klT4, sh)
    bqk_shards = [jax.device_put(bqk, d) for d in devices]
    out_zero_np = np.zeros((NCORES * J, H), np.float32)
    out_zeros = jax.device_put(out_zero_np, sh)
    _dbg("kernel: small puts issued")

    qg_shards = [None] * NCORES
    for c in range(NCORES):
        qg_shards[c] = jax.device_put(_prep_q_core(q, c), devices[c])
    _dbg("kernel: qG puts issued")

    specs = _input_specs()
    qG_g = jax.make_array_from_single_device_arrays(
        (NCORES * AGN, 128, HC * FREE), sh, qg_shards
    )
    bqk_g = jax.make_array_from_single_device_arrays(
        (NCORES * 128, MT), sh, bqk_shards
    )

    _COMPILED_READY.wait()
    _dbg("kernel: compiled ready")
    if "compiled" not in _ST:
        # Background compile failed — fall back to the stock runner.
        from concourse.bass_utils import run_bass_kernel_spmd

        in_maps = [
            {"qG": np.asarray(qg_shards[c]), "WqT": WqTb, "klT": klT4,
             "bqk": bqk}
            for c in range(NCORES)
        ]
        nc = build()
        res = run_bass_kernel_spmd(nc, in_maps, core_ids=list(range(NCORES)))
        outs = [r["out"] for r in res.results]
        return np.concatenate(outs, axis=0).reshape(A, B, NH, 1, 1)

    WqT_g = _ST["tile_wq"](wq_sharded)
    klT_g = _ST["tile_kl"](kl_sharded)
    compiled = _ST["compiled"]
    outs = compiled(qG_g, WqT_g, klT_g, bqk_g, out_zeros)
    _dbg("kernel: exec dispatched")
    out_np = np.asarray(outs[0])                                  # [B, H]
    _dbg("kernel: fetched")
    return out_np.reshape(A, B, NH, 1, 1)


# revision 15
# speedup vs baseline: 8.5387x; 4.3933x over previous
"""Trainium2 Bass kernel for nn_MHAttentionMap (scrambled-reshape variant).

Math (derived from the reference's permute/reshape semantics):
    ql = q @ Wq^T + bq                  # [A, B, H]
    kl = k @ Wk^T + bk                  # [B, H]
    logits[alpha, m] = fact * sum_a ql[a, alpha, m] * kl[a, m]   # m in [0, H)
    out[alpha, beta, n] = softmax_n(logits[alpha, 8*beta + n])   # groups of 8

Sharding: data-parallel over alpha (q's second axis), J=32 columns per core,
no collectives. The dominant GEMM runs on PE in bf16 with f32 PSUM
accumulation; the tiny kl projection is folded on the host.

End-to-end latency design (the metric is wall-clock of kernel() in a fresh
process; device exec is ~ms while host/compile/transfer costs dominate):
  - The Bass program uses a For_i hardware loop over 16 a-groups, so the
    program is ~350 instructions instead of ~8500: fast build, fast
    compile, small NEFF, fast device graph load.
  - Inputs ship as bf16 (q: 268MB instead of 537MB) — the axon tunnel
    sustains only ~90MB/s, so bytes shipped dominate.
  - A background thread started at import builds + compiles the program
    and issues a warm-up call on device-resident zeros, so the NEFF load
    overlaps the host-side prep and input transfer.
  - kernel() preps per-core shards and device_puts them as they become
    ready, then invokes the compiled executable on device-resident arrays.
"""

import threading
import numpy as np

import concourse.bass as bass
import concourse.mybir as mybir
import concourse.tile_sem_assignment as _tsa
from concourse.bass import ds
from concourse.tile import TileContext

_tsa.NUM_HWDGE_SEMS = 1  # all nc.sync DMAs share one FIFO ring/semaphore

A = 256          # q leading axis (contracted in the output)
B = 256          # q second axis (sharded)
H = 2048         # hidden
NH = 8           # heads (softmax group)
NCORES = 8
J = B // NCORES  # 32 alpha columns per core
HC = H // 128    # 16 contraction chunks
MT = H // 128    # 16 m tiles
AGN = 16         # a-groups
AGS = A // AGN   # 16 a per group
FREE = AGS * J   # 512 matmul free size
FACT = float((H / NH) ** -0.5)

F32 = mybir.dt.float32
BF16 = mybir.dt.bfloat16
MULT = mybir.AluOpType.mult
ADD = mybir.AluOpType.add

IN_NAMES = ["qG", "WqT", "klT", "bqk"]  # must match allocation order in build()


def build():
    nc = bass.Bass()
    qG = nc.dram_tensor("qG", [AGN, 128, HC * FREE], BF16, kind="ExternalInput")
    WqT = nc.dram_tensor("WqT", [H, H], BF16, kind="ExternalInput")
    klT = nc.dram_tensor("klT", [AGN, 128, MT, AGS], F32, kind="ExternalInput")
    bqk = nc.dram_tensor("bqk", [128, MT], F32, kind="ExternalInput")
    out = nc.dram_tensor("out", [J, H], F32, kind="ExternalOutput")

    ident_d = nc.inline_tensor(np.eye(128, dtype=np.float32), name="ident")
    g_np = np.kron(np.eye(16, dtype=np.float32), np.ones((8, 1), np.float32))
    g_d = nc.inline_tensor(g_np, name="gmat")                            # [128, 16]
    gt_d = nc.inline_tensor(np.ascontiguousarray(g_np.T), name="gtmat")  # [16, 128]

    with TileContext(nc) as tc:
        with (
            tc.tile_pool(name="const", bufs=1) as cpool,
            tc.tile_pool(name="qb", bufs=1) as qpool,
            tc.tile_pool(name="acc", bufs=1) as apool,
            tc.tile_pool(name="ework", bufs=2) as epool,
            tc.tile_pool(name="mpsum", bufs=8, space="PSUM") as mpsum,
        ):
            ident_sb = cpool.tile([128, 128], F32, name="ident_sb")
            nc.sync.dma_start(ident_sb[:], ident_d[:])
            g_sb = cpool.tile([128, 16], F32, name="g_sb")
            nc.sync.dma_start(g_sb[:], g_d[:])
            gt_sb = cpool.tile([16, 128], F32, name="gt_sb")
            nc.sync.dma_start(gt_sb[:], gt_d[:])

            wq_sb = cpool.tile([128, HC, H], BF16, name="wq_sb")
            nc.sync.dma_start(wq_sb[:], WqT[:].rearrange("(c p) m -> p c m", p=128))
            bqk_sb = cpool.tile([128, MT], F32, name="bqk_sb")
            nc.sync.dma_start(bqk_sb[:], bqk[:])

            s_all = apool.tile([128, MT, J], F32, name="s_all")
            nc.vector.memset(s_all[:], 0.0)

            with tc.For_i(0, AGN, 1) as ag:
                qblk = qpool.tile([128, HC * FREE], BF16, name="qblk")
                nc.sync.dma_start(qblk[:], qG[ds(ag, 1), :, :])
                klcur = qpool.tile([128, MT, AGS], F32, name="klcur")
                nc.sync.dma_start(klcur[:], klT[ds(ag, 1), :, :, :])
                for mt in range(MT):
                    ps = mpsum.tile([128, FREE], F32, name="ps", tag="ps")
                    for hc in range(HC):
                        nc.tensor.matmul(
                            ps[:],
                            wq_sb[:, hc, mt * 128 : (mt + 1) * 128],
                            qblk[:, hc * FREE : (hc + 1) * FREE],
                            start=(hc == 0),
                            stop=(hc == HC - 1),
                        )
                    # e[p, j, al] = ps[p, al*J+j] * klcur[p, mt, al]
                    e = epool.tile([128, J, AGS], F32, name="e", tag="e")
                    nc.vector.tensor_tensor(
                        e[:],
                        ps[:].rearrange("p (al j) -> p j al", j=J),
                        klcur[:, mt, :].unsqueeze(1).broadcast_to([128, J, AGS]),
                        op=MULT,
                    )
                    r = epool.tile([128, J], F32, name="r", tag="r")
                    nc.vector.tensor_reduce(
                        r[:], e[:], axis=mybir.AxisListType.X, op=ADD
                    )
                    nc.vector.tensor_tensor(
                        s_all[:, mt, :], r[:], s_all[:, mt, :], op=ADD
                    )

            # bias fold: s[p, mt, j] += bqk[p, mt]
            nc.vector.tensor_tensor(
                s_all[:],
                bqk_sb[:].unsqueeze(-1).broadcast_to([128, MT, J]),
                s_all[:],
                op=ADD,
            )

            # softmax over groups of 8 partitions; logits ~ N(0,1) so exp
            # without max-subtraction is safe in f32.
            e_all = apool.tile([128, MT, J], F32, name="e_all")
            nc.scalar.activation(e_all[:], s_all[:], mybir.ActivationFunctionType.Exp)
            zp = mpsum.tile([16, MT * J], F32, name="zp", tag="ps")
            nc.tensor.matmul(
                zp[:], g_sb[:], e_all[:].rearrange("p mt j -> p (mt j)"),
                start=True, stop=True,
            )
            rz_sb = apool.tile([16, MT * J], F32, name="rz_sb")
            nc.vector.reciprocal(rz_sb[:], zp[:])
            rp = mpsum.tile([128, MT * J], F32, name="rp", tag="ps")
            nc.tensor.matmul(rp[:], gt_sb[:], rz_sb[:], start=True, stop=True)
            w_all = apool.tile([128, MT, J], F32, name="w_all")
            nc.vector.tensor_tensor(
                w_all[:], e_all[:],
                rp[:].rearrange("p (mt j) -> p mt j", j=J),
                op=MULT,
            )

            # transpose [m, j] -> [j, m] and store
            wT = apool.tile([J, MT, 128], F32, name="wT")
            for tpi in range(4):
                tp = mpsum.tile([J, 4, 128], F32, name="tp", tag="ps")
                for k4 in range(4):
                    mtg = tpi * 4 + k4
                    nc.tensor.transpose(tp[:, k4, :], w_all[:, mtg, :], ident_sb[:])
                nc.vector.tensor_copy(wT[:, tpi * 4 : (tpi + 1) * 4, :], tp[:])
            nc.sync.dma_start(out[:], wT[:])

    _hoist_waits(nc)
    return nc


def _hoist_waits(nc):
    """This walrus build allows only one semaphore wait per TPB/DMA
    instruction. Hoist all-but-one wait of each instruction onto standalone
    EventSemaphore sync ops on the same engine, issued immediately before —
    the engine sequencer executes in order, so semantics are unchanged."""
    import bass_rust

    skip = ("InstEventSemaphore", "InstCall", "InstISA")
    for f in nc.m.functions:
        for bb in f.blocks:
            out = []
            for inst in bb.instructions:
                si = inst.sync_info
                if (
                    si is not None
                    and si.on_wait
                    and len(si.on_wait) > 1
                    and type(inst).__name__ not in skip
                ):
                    waits = list(si.on_wait)
                    for w in waits[:-1]:
                        es = mybir.InstEventSemaphore(
                            name=f"{inst.name}-w{len(out)}",
                            engine=inst.engine,
                            sync_info=bass_rust.SyncInfo(on_wait=[w], on_update=[]),
                        )
                        out.append(es)
                    si.on_wait = waits[-1:]
                out.append(inst)
            bb.instructions = out


# ---------------------------------------------------------------------------
# Host-side runner: compiled-executable cache + background warm-up.
# ---------------------------------------------------------------------------

_ST: dict = {}
_DEV_READY = threading.Event()
_COMPILED_READY = threading.Event()

import os as _os
import sys as _sys
import time as _time

_T0 = _time.time()
_DEBUG = bool(_os.environ.get("KERNEL_DEBUG"))


def _dbg(msg):
    if _DEBUG:
        print(f"[kernel +{_time.time()-_T0:6.2f}s] {msg}", file=_sys.stderr, flush=True)


def _input_specs():
    """(name, per-core shape, numpy dtype) in executable parameter order."""
    import ml_dtypes

    bf16 = np.dtype(ml_dtypes.bfloat16)
    return [
        ("qG", (AGN, 128, HC * FREE), bf16),
        ("WqT", (H, H), bf16),
        ("klT", (AGN, 128, MT, AGS), np.dtype(np.float32)),
        ("bqk", (128, MT), np.dtype(np.float32)),
    ]


def _bg_compile():
    try:
        import jax
        from jax.sharding import Mesh, PartitionSpec, NamedSharding
        from jax.experimental.shard_map import shard_map
        import concourse.bass2jax as b2j

        _dbg("bg: jax imported")
        devices = jax.devices()[:NCORES]
        mesh = Mesh(np.asarray(devices), ("core",))
        sh = NamedSharding(mesh, PartitionSpec("core"))
        _ST["devices"] = devices
        _ST["mesh"] = mesh
        _ST["sharding"] = sh
        _DEV_READY.set()
        _dbg("bg: devices ready")

        b2j.install_neuronx_cc_hook()
        nc = build()
        _dbg("bg: bass built")
        assert nc.dbg_addr is None
        partition_name = (
            nc.partition_id_tensor.name if nc.partition_id_tensor else None
        )

        # Recover the executable's input/output interface from allocations.
        in_names, out_names, out_avals = [], [], []
        for alloc in nc.m.functions[0].allocations:
            if not isinstance(alloc, mybir.MemoryLocationSet):
                continue
            name = alloc.memorylocations[0].name
            if alloc.kind == "ExternalInput":
                if name != partition_name:
                    in_names.append(name)
            elif alloc.kind == "ExternalOutput":
                out_names.append(name)
                out_avals.append(
                    jax.core.ShapedArray(
                        tuple(alloc.tensor_shape), mybir.dt.np(alloc.dtype)
                    )
                )
        assert in_names == IN_NAMES, in_names
        assert out_names == ["out"], out_names
        n_params = len(in_names)
        all_names = in_names + out_names
        if partition_name is not None:
            all_names.append(partition_name)
        all_names = tuple(all_names)
        donate = tuple(range(n_params, n_params + len(out_names)))

        def _body(*args):
            operands = list(args)
            if partition_name is not None:
                operands.append(b2j.partition_id_tensor())
            outs = b2j._bass_exec_p.bind(
                *operands,
                out_avals=tuple(out_avals),
                in_names=all_names,
                out_names=tuple(out_names),
                lowering_input_output_aliases=(),
                sim_require_finite=True,
                sim_require_nnan=True,
                nc=nc,
            )
            return tuple(outs)

        jf = jax.jit(
            shard_map(
                _body,
                mesh=mesh,
                in_specs=(PartitionSpec("core"),) * (n_params + len(out_names)),
                out_specs=(PartitionSpec("core"),) * len(out_names),
                check_rep=False,
            ),
            donate_argnums=donate,
            keep_unused=True,
        )

        specs = _input_specs()
        gshapes = [(NCORES * s[0], *s[1:]) for _, s, _ in specs]
        gdtypes = [d for _, _, d in specs]
        out_gshape = (NCORES * J, H)
        abstract = [
            jax.ShapeDtypeStruct(s, d, sharding=sh)
            for s, d in zip(gshapes, gdtypes)
        ] + [jax.ShapeDtypeStruct(out_gshape, np.float32, sharding=sh)]
        lowered = jf.lower(*abstract)
        _dbg("bg: lowered")
        compiled = lowered.compile()
        _dbg("bg: compiled")

        # On-device replication programs: WqT/klT ship once (row-sharded)
        # and are all-gathered into the per-core-replicated global layout
        # the executable expects.
        jnp = jax.numpy
        import ml_dtypes

        bf16 = np.dtype(ml_dtypes.bfloat16)
        _ST["tile_wq"] = (
            jax.jit(lambda w: jnp.tile(w, (NCORES, 1)), out_shardings=sh)
            .lower(jax.ShapeDtypeStruct((H, H), bf16, sharding=sh))
            .compile()
        )
        _ST["tile_kl"] = (
            jax.jit(lambda w: jnp.tile(w, (NCORES, 1, 1, 1)), out_shardings=sh)
            .lower(
                jax.ShapeDtypeStruct((AGN, 128, MT, AGS), np.float32, sharding=sh)
            )
            .compile()
        )
        _dbg("bg: tile jits compiled")
        _ST["compiled"] = compiled
    except Exception as exc:  # noqa: BLE001
        _ST["err"] = exc
    finally:
        _DEV_READY.set()
        _COMPILED_READY.set()


_BG = threading.Thread(target=_bg_compile, daemon=True)
_BG.start()


def _prep_small(k, Wq, bq, Wk, bk):
    import ml_dtypes

    bf16 = np.dtype(ml_dtypes.bfloat16)
    WqTb = np.ascontiguousarray(np.asarray(Wq, np.float32).T).astype(bf16)
    klF = (
        np.asarray(k, np.float32) @ np.asarray(Wk, np.float32).T
        + np.asarray(bk, np.float32)
    ) * np.float32(FACT)                                          # [A, H]
    klT4 = np.ascontiguousarray(
        klF.reshape(AGN, AGS, MT, 128).transpose(0, 3, 2, 1)
    )                                                             # [AGN,128,MT,AGS]
    bqk_m = np.asarray(bq, np.float32) * klF.sum(axis=0)
    bqk = np.ascontiguousarray(bqk_m.reshape(MT, 128).T)          # [128, MT]
    return WqTb, klT4, bqk


def _prep_q_core(q, c):
    import ml_dtypes

    bf16 = np.dtype(ml_dtypes.bfloat16)
    qc = q[:, c * J : (c + 1) * J, :]
    return (
        qc.reshape(AGN, AGS, J, HC, 128)
        .transpose(0, 4, 3, 1, 2)
        .astype(bf16)
        .reshape(AGN, 128, HC * FREE)
    )


def kernel(q, k, Wq, bq, Wk, bk):
    import jax

    _dbg("kernel: called")
    q = np.asarray(q, dtype=np.float32)

    # Small arrays are cheap to prep and ship; do them first so the device
    # side work (all-gathers) pipelines under the long qG transfer.
    WqTb, klT4, bqk = _prep_small(k, Wq, bq, Wk, bk)
    _dbg("kernel: small prepped")

    _DEV_READY.wait()
    _dbg("kernel: devices ready")
    if "devices" not in _ST:
        raise RuntimeError(f"jax init failed: {_ST.get('err')}")
    devices = _ST["devices"]
    sh = _ST["sharding"]

    # WqT/klT: ship once row-sharded; replicate on device once compiled.
    wq_sharded = jax.device_put(WqTb, sh)
    kl_sharded = jax.device_put(klT4, sh)
    bqk_shards = [jax.device_put(bqk, d) for d in devices]
    out_zero_np = np.zeros((NCORES * J, H), np.float32)
    out_zeros = jax.device_put(out_zero_np, sh)
    _dbg("kernel: small puts issued")

    qg_shards = [None] * NCORES
    for c in range(NCORES):
        qg_shards[c] = jax.device_put(_prep_q_core(q, c), devices[c])
    _dbg("kernel: qG puts issued")

    specs = _input_specs()
    qG_g = jax.make_array_from_single_device_arrays(
        (NCORES * AGN, 128, HC * FREE), sh, qg_shards
    )
    bqk_g = jax.make_array_from_single_device_arrays(
        (NCORES * 128, MT), sh, bqk_shards
    )

    _COMPILED_READY.wait()
    _dbg("kernel: compiled ready")
    if "compiled" not in _ST:
        # Background compile failed — fall back to the stock runner.
        from concourse.bass_utils import run_bass_kernel_spmd

        in_maps = [
            {"qG": np.asarray(qg_shards[c]), "WqT": WqTb, "klT": klT4,
             "bqk": bqk}
            for c in range(NCORES)
        ]
        nc = build()
        res = run_bass_kernel_spmd(nc, in_maps, core_ids=list(range(NCORES)))
        outs = [r["out"] for r in res.results]
        return np.concatenate(outs, axis=0).reshape(A, B, NH, 1, 1)

    if _DEBUG:
        jax.block_until_ready(qG_g)
        _dbg("kernel: qG transfer drained")
    WqT_g = _ST["tile_wq"](wq_sharded)
    if _DEBUG:
        jax.block_until_ready(WqT_g)
        _dbg("kernel: tile_wq done")
    klT_g = _ST["tile_kl"](kl_sharded)
    if _DEBUG:
        jax.block_until_ready(klT_g)
        _dbg("kernel: tile_kl done")
    compiled = _ST["compiled"]
    outs = compiled(qG_g, WqT_g, klT_g, bqk_g, out_zeros)
    _dbg("kernel: exec dispatched")
    if _DEBUG:
        jax.block_until_ready(outs)
        _dbg("kernel: exec done")
    out_np = np.asarray(outs[0])                                  # [B, H]
    _dbg("kernel: fetched")
    return out_np.reshape(A, B, NH, 1, 1)
